# revision 21
# baseline (speedup 1.0000x reference)
"""Gemma4 MoE feed-forward on 8 Trainium2 NeuronCores.

Strategy: expert-parallel. E == n_cores == 8, so core e owns expert e's
weights (Wg[e], Wu[e], Wd[e]) and receives exactly the tokens routed to
expert e (gathered + transposed + padded on the host). Each core runs a
dense gated-FFN over its token batch:

    dT = Wd^T @ (gelu_tanh(Wg^T x^T) * (Wu^T x^T))        (all [*, C] layouts)

The host then scatter-adds routing_weight * dT^T back into the full
[T, H] output. Tokens that select the same expert in both slots are
deduplicated on the host (weights summed).

All matmul I/O is bf16 (fp32 PSUM accumulation): bf16 runs the PE at the
same full rate as fp32r but halves every DMA byte, which matters because
the fp32 version of this kernel saturated the ~305 GB/s dynamic-DMA
fabric for 94% of the kernel and throttled the matmul cadence.

Loop order is i-outer / n-inner so each up-projection weight tile is
DMA'd exactly once (the n-outer variant re-fetched all 16.8 MB of
Wg/Wu per token-block sweep).
"""

import os
import sys

import numpy as np

for _p in ("/opt/trn_rl_repo", "/root/.axon_site/_ro/trn_rl_repo"):
    if os.path.isdir(_p) and _p not in sys.path:
        sys.path.append(_p)

T, H, I, E, K = 4096, 2048, 1024, 8, 2
NCORES = 8

# 'bf16' (default): bf16 data + matmul, fp32 PSUM accumulate
# 'f32r': fp32 data, relaxed-precision full-rate matmul
MM_MODE = os.environ.get("MOE_MM_MODE", "bf16")

_PROGRAM_CACHE = {}
LAST_RESULT = None  # BassKernelResults of the most recent run (for test.py)
TRACE = False  # test.py sets this to capture an NTFF profile
TRACE_CORES = [0]

# k-tiles (128 rows each) per weight DMA. 4KB contiguous bytes per
# partition doubles per-packet DMA efficiency vs 2KB (~97ns/packet fixed
# cost), lifting aggregate fabric throughput from ~330 to ~420 GB/s.
GUP = 16  # up-phase: one DMA per (matrix, i) = 4KB/partition in bf16
GDN = 8  # down-phase: one DMA per h = 2KB/partition in bf16

KH = H // 128  # 16 k-tiles over the hidden dim
KI = I // 128  # 8 k-tiles over the intermediate dim
GU = KH // GUP  # weight-DMA groups per i-tile (up phase): 1
GD = KI // GDN  # weight-DMA groups per h-tile (down phase): 1


def _round_fp32r(a):
    """Round fp32 to the FP32R format the PE consumes: 11-bit mantissa."""
    b = np.ascontiguousarray(a, dtype=np.float32).view(np.uint32)
    lsb = (b >> 12) & 1
    r = (b + 0x7FF + lsb) & 0xFFFFF000
    return r.view(np.float32)


def _tile_w_up(W, io_np):
    """[H, I] -> [KI, GU, 128, GUP*128]: tile (k,i) of W at
    [i, k//GUP, :, (k%GUP)*128:], so each (i, g) DMA reads
    GUP*128*itemsize contiguous bytes per partition."""
    Wt = W.reshape(KH // GUP, GUP, 128, KI, 128).transpose(3, 0, 2, 1, 4)
    return np.ascontiguousarray(Wt, dtype=io_np).reshape(KI, GU, 128, GUP * 128)


def _tile_w_down(W, io_np):
    """[I, H] -> [KH, GD, 128, GDN*128] (same scheme, contraction over I)."""
    Wt = W.reshape(KI // GDN, GDN, 128, KH, 128).transpose(3, 0, 2, 1, 4)
    return np.ascontiguousarray(Wt, dtype=io_np).reshape(KH, GD, 128, GDN * 128)


def _pick_config(max_count):
    """Minimal uniform token-block config: NT blocks of even width N with
    NT*N >= max_count, N <= 512 (PSUM bank limit) and N >= 256 (full-rate
    floor for the PE's moving operand)."""
    mc = max(max_count, 256)
    nt = -(-mc // 512)
    n = -(-mc // nt)
    n += n % 2
    return (nt * n, nt, n)  # (C, NT, N)


def _build_program(C, NT, N, mode):
    import concourse.tile as tile
    from concourse import bacc, mybir
    from contextlib import ExitStack

    assert NT == 2, "x pairing below assumes two token blocks"
    assert KH % 2 == 0

    f32 = mybir.dt.float32
    if mode == "f32r":
        io_dt = mybir.dt.float32r
    elif mode == "bf16":
        io_dt = mybir.dt.bfloat16
    else:
        io_dt = f32
    isz = 2 if mode == "bf16" else 4

    nc = bacc.Bacc("TRN2", target_bir_lowering=False, debug=False)

    # x arrives k-quadded and split by token half: xA[j] holds k-tiles
    # (4j..4j+3) columns [0:N), xB[j] the same k-tiles columns [N:2N).
    # One DMA per (j, half) = 4*N*isz contiguous bytes per partition.
    xA_d = nc.dram_tensor("xA", [KH // 4, 128, 4 * N], io_dt, kind="ExternalInput").ap()
    xB_d = nc.dram_tensor("xB", [KH // 4, 128, 4 * N], io_dt, kind="ExternalInput").ap()
    Wg_d = nc.dram_tensor("Wg", [KI, GU, 128, GUP * 128], io_dt, kind="ExternalInput").ap()
    Wu_d = nc.dram_tensor("Wu", [KI, GU, 128, GUP * 128], io_dt, kind="ExternalInput").ap()
    Wd_d = nc.dram_tensor("Wd", [KH, GD, 128, GDN * 128], io_dt, kind="ExternalInput").ap()
    dT = nc.dram_tensor("dT", [H, C], io_dt, kind="ExternalOutput").ap()

    dT_p = dT.rearrange("(a p) c -> p a c", p=128)  # [128, KH, C]

    GELU = mybir.ActivationFunctionType.Gelu_apprx_tanh

    with tile.TileContext(nc) as tc, ExitStack() as ctx:
        # SBUF/partition: x 8*4N*isz + aT KI*C*isz + w 4 tags*3*2KB
        # + wd 4*2KB + gel 4*N*4 + o 4*N*4 + warm ~1KB  (~97KB for bf16)
        xpool = ctx.enter_context(tc.tile_pool(name="x", bufs=1))
        wpool = ctx.enter_context(tc.tile_pool(name="w", bufs=3))
        apool = ctx.enter_context(tc.tile_pool(name="a", bufs=1))
        tpool = ctx.enter_context(tc.tile_pool(name="t", bufs=4))
        opool = ctx.enter_context(tc.tile_pool(name="o", bufs=4))
        wdpool = ctx.enter_context(tc.tile_pool(name="wd", bufs=4))

        # PE clock-gate warmup: HAM starts at 1.2 GHz and un-throttles only
        # after ~3.4us of sustained activity. Real matmuls are DMA-gated at
        # launch; dummy bf16 matmuls on memset scratch need no DMA, so they
        # run immediately and the real stream begins at 2.4 GHz. The memset
        # runs on GpSimd, whose queue is free ~0.5us before Vector's.
        with (
            tc.tile_pool(name="warm", bufs=1) as wmpool,
            tc.tile_pool(name="warmps", bufs=1, space="PSUM") as wmpspool,
        ):
            wt = wmpool.tile([128, 128], mybir.dt.bfloat16, name="warm_in")
            nc.gpsimd.memset(wt[:], 0.0)
            wps = wmpspool.tile([128, 128], f32, name="warm_ps")
            # sized to span until the first x/weight DMAs land (~5.5-7.5us
            # after PE start, run-to-run variable): undershooting by >3.4us
            # lets HAM re-throttle, which costs ~2.6us of cold matmuls
            for r in range(30):
                nc.tensor.matmul(wps[:], wt[:], wt[:], start=True, stop=True)

        w_tiles = {}

        # DMA descriptors cost ~0.6us of issue time on the emitting engine,
        # so they are spread across both HWDGE rings: Wg on the Sync ring,
        # Wu on the Scalar ring (and x alternates below).
        def issue_w(i):
            wgt = wpool.tile([128, GUP * 128], io_dt, tag="wg", name=f"wg{i}")
            wut = wpool.tile([128, GUP * 128], io_dt, tag="wu", name=f"wu{i}")
            nc.sync.dma_start(wgt[:], Wg_d[i, 0])
            nc.scalar.dma_start(wut[:], Wu_d[i, 0])
            w_tiles[i] = ([wgt], [wut])

        wd_tiles = {}

        def issue_wd(h):
            wd_gs = []
            for g in range(GD):
                wdt = wdpool.tile([128, GDN * 128], io_dt, tag=f"wd{g}", name=f"wd{h}_{g}")
                nc.sync.dma_start(wdt[:], Wd_d[h, g])
                wd_gs.append(wdt)
            wd_tiles[h] = wd_gs

        # Critical-path emission order. The two HWDGE rings drain in FIFO
        # order at ~100-200GB/s each, so the first matmul's data (wg k0-7,
        # wu k0-7, xA0) must sit at the FRONT of each ring: the i=0 weight
        # halves interleave with the x quads, the rest of x follows by
        # need-time, and i=1 weights come last (still ~3us early at ~400
        # GB/s aggregate). Afterwards weights prefetch 2 i-iterations
        # ahead, far off the critical path.
        xts = [xpool.tile([128, 8 * N], io_dt, name=f"xt{j}") for j in range(KH // 4)]

        def issue_x(j, half):
            eng = nc.sync if j % 2 == 0 else nc.scalar
            src = xA_d if half == 0 else xB_d
            eng.dma_start(xts[j][:, 4 * N * half : 4 * N * (half + 1)], src[j])

        wgt0 = wpool.tile([128, GUP * 128], io_dt, tag="wg", name="wg0")
        wut0 = wpool.tile([128, GUP * 128], io_dt, tag="wu", name="wu0")
        hw = GUP * 64  # columns holding k-tiles 0..7
        nc.sync.dma_start(wgt0[:, 0:hw], Wg_d[0, 0, :, 0:hw])
        nc.scalar.dma_start(wut0[:, 0:hw], Wu_d[0, 0, :, 0:hw])
        issue_x(0, 0)  # sync: k0-3 first-half tokens
        issue_x(1, 0)  # scalar: k4-7
        nc.sync.dma_start(wgt0[:, hw:], Wg_d[0, 0, :, hw:])
        nc.scalar.dma_start(wut0[:, hw:], Wu_d[0, 0, :, hw:])
        w_tiles[0] = ([wgt0], [wut0])
        issue_x(2, 0)
        issue_x(3, 0)
        for j in range(KH // 4):
            issue_x(j, 1)
        issue_w(1)

        def xsl(k, n):
            # column slice of xts[k//4] holding k-tile k, token-block n
            base = (4 * n + (k % 4)) * N
            return xts[k // 4][:, base : base + N]

        aT = apool.tile([128, KI, C], io_dt, name="aT")

        # Both PSUM pools are opened once for the whole kernel (2+2 tags
        # * 2 bufs = 8 banks): closing gu before opening d would insert a
        # drain barrier (~1.5us of PE idle at the up->down transition).
        gupool = ctx.enter_context(tc.tile_pool(name="gu", bufs=2, space="PSUM"))
        dpool = ctx.enter_context(tc.tile_pool(name="d", bufs=2, space="PSUM"))

        for i in range(KI):
            if i + 2 < KI and i + 2 not in w_tiles:
                issue_w(i + 2)
            # prefetch the first down-phase weights near the end of up
            if i >= KI - 3 and (h := i - (KI - 3)) < 3:
                issue_wd(h)
            wg_gs, wu_gs = w_tiles.pop(i)
            for n in range(NT):
                g_ps = gupool.tile([128, N], f32, tag="g", name=f"g{i}_{n}")
                u_ps = gupool.tile([128, N], f32, tag="u", name=f"u{i}_{n}")
                for k in range(KH):
                    ksl = slice((k % GUP) * 128, (k % GUP + 1) * 128)
                    nc.tensor.matmul(
                        g_ps[:],
                        wg_gs[k // GUP][:, ksl],
                        xsl(k, n),
                        start=(k == 0),
                        stop=(k == KH - 1),
                    )
                    nc.tensor.matmul(
                        u_ps[:],
                        wu_gs[k // GUP][:, ksl],
                        xsl(k, n),
                        start=(k == 0),
                        stop=(k == KH - 1),
                    )
                nsl = slice(n * N, (n + 1) * N)
                gel = tpool.tile([128, N], f32, tag="gelu", name=f"gel{i}_{n}")
                nc.scalar.activation(gel[:], g_ps[:], GELU)
                nc.vector.tensor_mul(aT[:, i, nsl], gel[:], u_ps[:])

        for h in range(KH):
            if h + 3 < KH and h + 3 not in wd_tiles:
                issue_wd(h + 3)
            if h not in wd_tiles:
                issue_wd(h)
            wd_gs = wd_tiles.pop(h)
            # n-outer: block n=0's PSUM drains + output DMA overlap block
            # n=1's accumulation, so only one short chain trails the final
            # matmul. n=0 outputs issue on the Scalar ring (idle in this
            # phase), n=1 on Sync; the last tile is split in half across
            # both rings so the tail drains in parallel.
            for n in range(NT):
                last = h == KH - 1 and n == NT - 1
                o = opool.tile([128, N], io_dt, tag=f"o{n}", name=f"o{h}_{n}")
                if not last:
                    d_ps = dpool.tile([128, N], f32, tag=f"d{n}", name=f"d{h}_{n}")
                    for ki in range(KI):
                        lw = wd_gs[ki // GDN][:, (ki % GDN) * 128 : (ki % GDN + 1) * 128]
                        nc.tensor.matmul(
                            d_ps[:],
                            lw,
                            aT[:, ki, n * N : (n + 1) * N],
                            start=(ki == 0),
                            stop=(ki == KI - 1),
                        )
                    eng = nc.scalar if n == 0 else nc.sync
                    nc.vector.tensor_copy(o[:], d_ps[:])
                    eng.dma_start(dT_p[:, h, n * N : (n + 1) * N], o[:])
                else:
                    # final tile: two half-width accumulation groups so the
                    # first half's cast+DMA overlaps the second half's
                    # matmuls, and the two DMAs issue on separate rings
                    hn = N // 2
                    for half, (eng, c0) in enumerate(
                        ((nc.sync, 0), (nc.scalar, hn))
                    ):
                        d_ps = dpool.tile(
                            [128, hn], f32, tag=f"d{n}", name=f"d{h}_{n}_{half}"
                        )
                        for ki in range(KI):
                            lw = wd_gs[ki // GDN][
                                :, (ki % GDN) * 128 : (ki % GDN + 1) * 128
                            ]
                            nc.tensor.matmul(
                                d_ps[:],
                                lw,
                                aT[:, ki, n * N + c0 : n * N + c0 + hn],
                                start=(ki == 0),
                                stop=(ki == KI - 1),
                            )
                        nc.vector.tensor_copy(o[:, c0 : c0 + hn], d_ps[:])
                        eng.dma_start(
                            dT_p[:, h, n * N + c0 : n * N + c0 + hn],
                            o[:, c0 : c0 + hn],
                        )

    nc.compile()
    return nc


def _get_program(C, NT, N, mode):
    key = (C, NT, N, mode)
    if key not in _PROGRAM_CACHE:
        _PROGRAM_CACHE[key] = _build_program(C, NT, N, mode)
    return _PROGRAM_CACHE[key]


def _ensure_ntff_hook():
    """Register the axon NTFF profile hook if the image's antenv lacks
    axon_hooks (see trn_agent_boot.trn_boot). Only needed when TRACE."""
    import types

    try:
        from antenv.axon_hooks import get_axon_ntff_profile_hook  # noqa: F401

        return
    except ImportError:
        pass
    import antenv
    from trn_agent_boot.trn_boot import _ntff_profile_via_ctypes

    hook = _ntff_profile_via_ctypes("/opt/axon/libaxon_pjrt.so")
    mod = types.ModuleType("antenv.axon_hooks")
    state = {"hook": hook}
    mod.set_axon_ntff_profile_hook = lambda h: state.__setitem__("hook", h)
    mod.get_axon_ntff_profile_hook = lambda: state["hook"]
    sys.modules["antenv.axon_hooks"] = mod
    antenv.axon_hooks = mod


def kernel(x, Wg, Wu, Wd, selected_experts, routing_weights):
    global LAST_RESULT
    from concourse.bass_utils import run_bass_kernel_spmd

    if TRACE:
        _ensure_ntff_hook()

    x = np.asarray(x, dtype=np.float32)
    Wg = np.asarray(Wg, dtype=np.float32)
    Wu = np.asarray(Wu, dtype=np.float32)
    Wd = np.asarray(Wd, dtype=np.float32)
    selected_experts = np.asarray(selected_experts)
    routing_weights = np.asarray(routing_weights, dtype=np.float32)

    # Host-side dispatch: per expert, the (deduplicated) token list and
    # summed routing weights.
    idx_list, w_list = [], []
    for e in range(E):
        m = selected_experts == e  # [T, K]
        idx = np.nonzero(m.any(axis=1))[0]
        w = (routing_weights * m).sum(axis=1)[idx]
        idx_list.append(idx)
        w_list.append(w.astype(np.float32))

    max_count = max(len(idx) for idx in idx_list)
    C, NT, N = _pick_config(max_count)

    mode = MM_MODE
    if mode == "bf16":
        import ml_dtypes

        io_np = ml_dtypes.bfloat16
        prep = lambda a: np.ascontiguousarray(a, dtype=io_np)
    elif mode == "f32r":
        io_np = np.float32
        prep = _round_fp32r
    else:
        io_np = np.float32
        prep = lambda a: np.ascontiguousarray(a, dtype=io_np)

    nc = _get_program(C, NT, N, mode)

    in_maps = []
    for e in range(E):
        idx = idx_list[e]
        xT = np.zeros((H, C), dtype=io_np)
        xT[:, : len(idx)] = prep(x[idx].T)
        # k-quadded token-half layout:
        # xA[j, p] = [x(4j)[0:N] | x(4j+1)[0:N] | x(4j+2)[0:N] | x(4j+3)[0:N]]
        xk = xT.reshape(KH, 128, C)
        xA = np.ascontiguousarray(
            xk[:, :, 0:N].reshape(KH // 4, 4, 128, N).transpose(0, 2, 1, 3)
        ).reshape(KH // 4, 128, 4 * N)
        xB = np.ascontiguousarray(
            xk[:, :, N : 2 * N].reshape(KH // 4, 4, 128, N).transpose(0, 2, 1, 3)
        ).reshape(KH // 4, 128, 4 * N)
        in_maps.append(
            {
                "xA": xA,
                "xB": xB,
                "Wg": _tile_w_up(prep(Wg[e]), io_np),
                "Wu": _tile_w_up(prep(Wu[e]), io_np),
                "Wd": _tile_w_down(prep(Wd[e]), io_np),
            }
        )

    res = run_bass_kernel_spmd(
        nc,
        in_maps,
        list(range(NCORES)),
        trace=TRACE,
        trace_cores=TRACE_CORES if TRACE else None,
    )
    LAST_RESULT = res

    out = np.zeros((T, H), dtype=np.float32)
    for e in range(E):
        idx = idx_list[e]
        dTe = np.asarray(res.results[e]["dT"], dtype=np.float32)  # [H, C]
        out[idx] += w_list[e][:, None] * dTe[:, : len(idx)].T
    return out


# revision 22
# speedup vs baseline: 1.0159x; 1.0159x over previous
"""Gemma4 MoE feed-forward on 8 Trainium2 NeuronCores.

Strategy: expert-parallel. E == n_cores == 8, so core e owns expert e's
weights (Wg[e], Wu[e], Wd[e]) and receives exactly the tokens routed to
expert e (gathered + transposed + padded on the host). Each core runs a
dense gated-FFN over its token batch:

    dT = Wd^T @ (gelu_tanh(Wg^T x^T) * (Wu^T x^T))        (all [*, C] layouts)

The host then scatter-adds routing_weight * dT^T back into the full
[T, H] output. Tokens that select the same expert in both slots are
deduplicated on the host (weights summed).

All matmul I/O is bf16 (fp32 PSUM accumulation): bf16 runs the PE at the
same full rate as fp32r but halves every DMA byte, which matters because
the fp32 version of this kernel saturated the ~305 GB/s dynamic-DMA
fabric for 94% of the kernel and throttled the matmul cadence.

Loop order is i-outer / n-inner so each up-projection weight tile is
DMA'd exactly once (the n-outer variant re-fetched all 16.8 MB of
Wg/Wu per token-block sweep).
"""

import os
import sys

import numpy as np

for _p in ("/opt/trn_rl_repo", "/root/.axon_site/_ro/trn_rl_repo"):
    if os.path.isdir(_p) and _p not in sys.path:
        sys.path.append(_p)

T, H, I, E, K = 4096, 2048, 1024, 8, 2
NCORES = 8

# 'bf16' (default): bf16 data + matmul, fp32 PSUM accumulate
# 'f32r': fp32 data, relaxed-precision full-rate matmul
MM_MODE = os.environ.get("MOE_MM_MODE", "bf16")

_PROGRAM_CACHE = {}
LAST_RESULT = None  # BassKernelResults of the most recent run (for test.py)
TRACE = False  # test.py sets this to capture an NTFF profile
TRACE_CORES = [0]

# k-tiles (128 rows each) per weight DMA. 4KB contiguous bytes per
# partition doubles per-packet DMA efficiency vs 2KB (~97ns/packet fixed
# cost), lifting aggregate fabric throughput from ~330 to ~420 GB/s.
GUP = 16  # up-phase: one DMA per (matrix, i) = 4KB/partition in bf16
GDN = 8  # down-phase: one DMA per h = 2KB/partition in bf16

KH = H // 128  # 16 k-tiles over the hidden dim
KI = I // 128  # 8 k-tiles over the intermediate dim
GU = KH // GUP  # weight-DMA groups per i-tile (up phase): 1
GD = KI // GDN  # weight-DMA groups per h-tile (down phase): 1


def _round_fp32r(a):
    """Round fp32 to the FP32R format the PE consumes: 11-bit mantissa."""
    b = np.ascontiguousarray(a, dtype=np.float32).view(np.uint32)
    lsb = (b >> 12) & 1
    r = (b + 0x7FF + lsb) & 0xFFFFF000
    return r.view(np.float32)


def _tile_w_up(W, io_np):
    """[H, I] -> [KI, GU, 128, GUP*128]: tile (k,i) of W at
    [i, k//GUP, :, (k%GUP)*128:], so each (i, g) DMA reads
    GUP*128*itemsize contiguous bytes per partition."""
    Wt = W.reshape(KH // GUP, GUP, 128, KI, 128).transpose(3, 0, 2, 1, 4)
    return np.ascontiguousarray(Wt, dtype=io_np).reshape(KI, GU, 128, GUP * 128)


def _tile_w_down(W, io_np):
    """[I, H] -> [KH, GD, 128, GDN*128] (same scheme, contraction over I)."""
    Wt = W.reshape(KI // GDN, GDN, 128, KH, 128).transpose(3, 0, 2, 1, 4)
    return np.ascontiguousarray(Wt, dtype=io_np).reshape(KH, GD, 128, GDN * 128)


def _pick_config(max_count):
    """Minimal uniform token-block config: NT blocks of even width N with
    NT*N >= max_count, N <= 512 (PSUM bank limit) and N >= 256 (full-rate
    floor for the PE's moving operand)."""
    mc = max(max_count, 256)
    nt = -(-mc // 512)
    n = -(-mc // nt)
    n += n % 2
    return (nt * n, nt, n)  # (C, NT, N)


def _build_program(C, NT, N, mode):
    import concourse.tile as tile
    from concourse import bacc, mybir
    from contextlib import ExitStack

    assert NT == 2, "x pairing below assumes two token blocks"
    assert KH % 2 == 0

    f32 = mybir.dt.float32
    if mode == "f32r":
        io_dt = mybir.dt.float32r
    elif mode == "bf16":
        io_dt = mybir.dt.bfloat16
    else:
        io_dt = f32
    isz = 2 if mode == "bf16" else 4

    nc = bacc.Bacc("TRN2", target_bir_lowering=False, debug=False)

    # x arrives k-quadded and split by token half: xA[j] holds k-tiles
    # (4j..4j+3) columns [0:N), xB[j] the same k-tiles columns [N:2N).
    # One DMA per (j, half) = 4*N*isz contiguous bytes per partition.
    xA_d = nc.dram_tensor("xA", [KH // 4, 128, 4 * N], io_dt, kind="ExternalInput").ap()
    xB_d = nc.dram_tensor("xB", [KH // 4, 128, 4 * N], io_dt, kind="ExternalInput").ap()
    Wg_d = nc.dram_tensor("Wg", [KI, GU, 128, GUP * 128], io_dt, kind="ExternalInput").ap()
    Wu_d = nc.dram_tensor("Wu", [KI, GU, 128, GUP * 128], io_dt, kind="ExternalInput").ap()
    Wd_d = nc.dram_tensor("Wd", [KH, GD, 128, GDN * 128], io_dt, kind="ExternalInput").ap()
    dT = nc.dram_tensor("dT", [H, C], io_dt, kind="ExternalOutput").ap()

    dT_p = dT.rearrange("(a p) c -> p a c", p=128)  # [128, KH, C]

    GELU = mybir.ActivationFunctionType.Gelu_apprx_tanh

    with tile.TileContext(nc) as tc, ExitStack() as ctx:
        # SBUF/partition: x 8*4N*isz + aT KI*C*isz + w 4 tags*3*2KB
        # + wd 4*2KB + gel 4*N*4 + o 4*N*4 + warm ~1KB  (~97KB for bf16)
        xpool = ctx.enter_context(tc.tile_pool(name="x", bufs=1))
        wpool = ctx.enter_context(tc.tile_pool(name="w", bufs=3))
        apool = ctx.enter_context(tc.tile_pool(name="a", bufs=1))
        tpool = ctx.enter_context(tc.tile_pool(name="t", bufs=4))
        opool = ctx.enter_context(tc.tile_pool(name="o", bufs=4))
        wdpool = ctx.enter_context(tc.tile_pool(name="wd", bufs=4))

        # PE clock-gate warmup: HAM starts at 1.2 GHz and un-throttles only
        # after ~3.4us of sustained activity. Real matmuls are DMA-gated at
        # launch; dummy bf16 matmuls on memset scratch need no DMA, so they
        # run immediately and the real stream begins at 2.4 GHz. The memset
        # runs on GpSimd, whose queue is free ~0.5us before Vector's.
        with (
            tc.tile_pool(name="warm", bufs=1) as wmpool,
            tc.tile_pool(name="warmps", bufs=1, space="PSUM") as wmpspool,
        ):
            wt = wmpool.tile([128, 512], mybir.dt.bfloat16, name="warm_in")
            nc.gpsimd.memset(wt[:], 0.0)
            wps = wmpspool.tile([128, 512], f32, name="warm_ps")
            # N=512 and count sized to SPAN ~5.6us, until the first
            # x/weight DMAs land (~4.5-7.5us after PE start, run-to-run
            # variable): ending >3.4us before the data arrives lets HAM
            # re-throttle, which costs ~2.6us of cold matmuls (measured —
            # shorter/faster warmup variants both lost ~3us this way)
            for r in range(18):
                nc.tensor.matmul(wps[:], wt[:, 0:128], wt[:], start=True, stop=True)

        w_tiles = {}

        # DMA descriptors cost ~0.6us of issue time on the emitting engine,
        # so they are spread across both HWDGE rings: Wg on the Sync ring,
        # Wu on the Scalar ring (and x alternates below).
        def issue_w(i):
            wgt = wpool.tile([128, GUP * 128], io_dt, tag="wg", name=f"wg{i}")
            wut = wpool.tile([128, GUP * 128], io_dt, tag="wu", name=f"wu{i}")
            nc.sync.dma_start(wgt[:], Wg_d[i, 0])
            nc.scalar.dma_start(wut[:], Wu_d[i, 0])
            w_tiles[i] = ([wgt], [wut])

        wd_tiles = {}

        def issue_wd(h):
            wd_gs = []
            for g in range(GD):
                wdt = wdpool.tile([128, GDN * 128], io_dt, tag=f"wd{g}", name=f"wd{h}_{g}")
                nc.sync.dma_start(wdt[:], Wd_d[h, g])
                wd_gs.append(wdt)
            wd_tiles[h] = wd_gs

        # Critical-path emission order. The two HWDGE rings drain in FIFO
        # order at ~100-200GB/s each, so the first matmul's data (wg k0-7,
        # wu k0-7, xA0) must sit at the FRONT of each ring: the i=0 weight
        # halves interleave with the x quads, the rest of x follows by
        # need-time, and i=1 weights come last (still ~3us early at ~400
        # GB/s aggregate). Afterwards weights prefetch 2 i-iterations
        # ahead, far off the critical path.
        xts = [xpool.tile([128, 8 * N], io_dt, name=f"xt{j}") for j in range(KH // 4)]

        def issue_x(j, half):
            eng = nc.sync if j % 2 == 0 else nc.scalar
            src = xA_d if half == 0 else xB_d
            eng.dma_start(xts[j][:, 4 * N * half : 4 * N * (half + 1)], src[j])

        wgt0 = wpool.tile([128, GUP * 128], io_dt, tag="wg", name="wg0")
        wut0 = wpool.tile([128, GUP * 128], io_dt, tag="wu", name="wu0")
        hw = GUP * 64  # columns holding k-tiles 0..7
        nc.sync.dma_start(wgt0[:, 0:hw], Wg_d[0, 0, :, 0:hw])
        nc.scalar.dma_start(wut0[:, 0:hw], Wu_d[0, 0, :, 0:hw])
        issue_x(0, 0)  # sync: k0-3 first-half tokens
        issue_x(1, 0)  # scalar: k4-7
        nc.sync.dma_start(wgt0[:, hw:], Wg_d[0, 0, :, hw:])
        nc.scalar.dma_start(wut0[:, hw:], Wu_d[0, 0, :, hw:])
        w_tiles[0] = ([wgt0], [wut0])
        issue_x(2, 0)
        issue_x(3, 0)
        for j in range(KH // 4):
            issue_x(j, 1)
        issue_w(1)

        def xsl(k, n):
            # column slice of xts[k//4] holding k-tile k, token-block n
            base = (4 * n + (k % 4)) * N
            return xts[k // 4][:, base : base + N]

        aT = apool.tile([128, KI, C], io_dt, name="aT")

        # Both PSUM pools are opened once for the whole kernel (2+2 tags
        # * 2 bufs = 8 banks): closing gu before opening d would insert a
        # drain barrier (~1.5us of PE idle at the up->down transition).
        gupool = ctx.enter_context(tc.tile_pool(name="gu", bufs=2, space="PSUM"))
        dpool = ctx.enter_context(tc.tile_pool(name="d", bufs=2, space="PSUM"))

        for i in range(KI):
            if i + 2 < KI and i + 2 not in w_tiles:
                issue_w(i + 2)
            # prefetch the first down-phase weights near the end of up
            if i >= KI - 3 and (h := i - (KI - 3)) < 3:
                issue_wd(h)
            wg_gs, wu_gs = w_tiles.pop(i)
            for n in range(NT):
                g_ps = gupool.tile([128, N], f32, tag="g", name=f"g{i}_{n}")
                u_ps = gupool.tile([128, N], f32, tag="u", name=f"u{i}_{n}")
                for k in range(KH):
                    ksl = slice((k % GUP) * 128, (k % GUP + 1) * 128)
                    nc.tensor.matmul(
                        g_ps[:],
                        wg_gs[k // GUP][:, ksl],
                        xsl(k, n),
                        start=(k == 0),
                        stop=(k == KH - 1),
                    )
                    nc.tensor.matmul(
                        u_ps[:],
                        wu_gs[k // GUP][:, ksl],
                        xsl(k, n),
                        start=(k == 0),
                        stop=(k == KH - 1),
                    )
                nsl = slice(n * N, (n + 1) * N)
                gel = tpool.tile([128, N], f32, tag="gelu", name=f"gel{i}_{n}")
                nc.scalar.activation(gel[:], g_ps[:], GELU)
                nc.vector.tensor_mul(aT[:, i, nsl], gel[:], u_ps[:])

        for h in range(KH):
            if h + 3 < KH and h + 3 not in wd_tiles:
                issue_wd(h + 3)
            if h not in wd_tiles:
                issue_wd(h)
            wd_gs = wd_tiles.pop(h)
            # n-outer: block n=0's PSUM drains + output DMA overlap block
            # n=1's accumulation, so only one short chain trails the final
            # matmul. n=0 outputs issue on the Scalar ring (idle in this
            # phase), n=1 on Sync; the last tile is split in half across
            # both rings so the tail drains in parallel.
            for n in range(NT):
                last = h == KH - 1 and n == NT - 1
                o = opool.tile([128, N], io_dt, tag=f"o{n}", name=f"o{h}_{n}")
                if not last:
                    d_ps = dpool.tile([128, N], f32, tag=f"d{n}", name=f"d{h}_{n}")
                    for ki in range(KI):
                        lw = wd_gs[ki // GDN][:, (ki % GDN) * 128 : (ki % GDN + 1) * 128]
                        nc.tensor.matmul(
                            d_ps[:],
                            lw,
                            aT[:, ki, n * N : (n + 1) * N],
                            start=(ki == 0),
                            stop=(ki == KI - 1),
                        )
                    eng = nc.scalar if n == 0 else nc.sync
                    nc.vector.tensor_copy(o[:], d_ps[:])
                    eng.dma_start(dT_p[:, h, n * N : (n + 1) * N], o[:])
                else:
                    # final tile: two half-width accumulation groups so the
                    # first half's cast+DMA overlaps the second half's
                    # matmuls, and the two DMAs issue on separate rings
                    hn = N // 2
                    for half, (eng, c0) in enumerate(
                        ((nc.sync, 0), (nc.scalar, hn))
                    ):
                        d_ps = dpool.tile(
                            [128, hn], f32, tag=f"d{n}", name=f"d{h}_{n}_{half}"
                        )
                        for ki in range(KI):
                            lw = wd_gs[ki // GDN][
                                :, (ki % GDN) * 128 : (ki % GDN + 1) * 128
                            ]
                            nc.tensor.matmul(
                                d_ps[:],
                                lw,
                                aT[:, ki, n * N + c0 : n * N + c0 + hn],
                                start=(ki == 0),
                                stop=(ki == KI - 1),
                            )
                        nc.vector.tensor_copy(o[:, c0 : c0 + hn], d_ps[:])
                        eng.dma_start(
                            dT_p[:, h, n * N + c0 : n * N + c0 + hn],
                            o[:, c0 : c0 + hn],
                        )

    nc.compile()
    return nc


def _get_program(C, NT, N, mode):
    key = (C, NT, N, mode)
    if key not in _PROGRAM_CACHE:
        _PROGRAM_CACHE[key] = _build_program(C, NT, N, mode)
    return _PROGRAM_CACHE[key]


def _ensure_ntff_hook():
    """Register the axon NTFF profile hook if the image's antenv lacks
    axon_hooks (see trn_agent_boot.trn_boot). Only needed when TRACE."""
    import types

    try:
        from antenv.axon_hooks import get_axon_ntff_profile_hook  # noqa: F401

        return
    except ImportError:
        pass
    import antenv
    from trn_agent_boot.trn_boot import _ntff_profile_via_ctypes

    hook = _ntff_profile_via_ctypes("/opt/axon/libaxon_pjrt.so")
    mod = types.ModuleType("antenv.axon_hooks")
    state = {"hook": hook}
    mod.set_axon_ntff_profile_hook = lambda h: state.__setitem__("hook", h)
    mod.get_axon_ntff_profile_hook = lambda: state["hook"]
    sys.modules["antenv.axon_hooks"] = mod
    antenv.axon_hooks = mod


def kernel(x, Wg, Wu, Wd, selected_experts, routing_weights):
    global LAST_RESULT
    from concourse.bass_utils import run_bass_kernel_spmd

    if TRACE:
        _ensure_ntff_hook()

    x = np.asarray(x, dtype=np.float32)
    Wg = np.asarray(Wg, dtype=np.float32)
    Wu = np.asarray(Wu, dtype=np.float32)
    Wd = np.asarray(Wd, dtype=np.float32)
    selected_experts = np.asarray(selected_experts)
    routing_weights = np.asarray(routing_weights, dtype=np.float32)

    # Host-side dispatch: per expert, the (deduplicated) token list and
    # summed routing weights.
    idx_list, w_list = [], []
    for e in range(E):
        m = selected_experts == e  # [T, K]
        idx = np.nonzero(m.any(axis=1))[0]
        w = (routing_weights * m).sum(axis=1)[idx]
        idx_list.append(idx)
        w_list.append(w.astype(np.float32))

    max_count = max(len(idx) for idx in idx_list)
    C, NT, N = _pick_config(max_count)

    mode = MM_MODE
    if mode == "bf16":
        import ml_dtypes

        io_np = ml_dtypes.bfloat16
        prep = lambda a: np.ascontiguousarray(a, dtype=io_np)
    elif mode == "f32r":
        io_np = np.float32
        prep = _round_fp32r
    else:
        io_np = np.float32
        prep = lambda a: np.ascontiguousarray(a, dtype=io_np)

    nc = _get_program(C, NT, N, mode)

    in_maps = []
    for e in range(E):
        idx = idx_list[e]
        xT = np.zeros((H, C), dtype=io_np)
        xT[:, : len(idx)] = prep(x[idx].T)
        # k-quadded token-half layout:
        # xA[j, p] = [x(4j)[0:N] | x(4j+1)[0:N] | x(4j+2)[0:N] | x(4j+3)[0:N]]
        xk = xT.reshape(KH, 128, C)
        xA = np.ascontiguousarray(
            xk[:, :, 0:N].reshape(KH // 4, 4, 128, N).transpose(0, 2, 1, 3)
        ).reshape(KH // 4, 128, 4 * N)
        xB = np.ascontiguousarray(
            xk[:, :, N : 2 * N].reshape(KH // 4, 4, 128, N).transpose(0, 2, 1, 3)
        ).reshape(KH // 4, 128, 4 * N)
        in_maps.append(
            {
                "xA": xA,
                "xB": xB,
                "Wg": _tile_w_up(prep(Wg[e]), io_np),
                "Wu": _tile_w_up(prep(Wu[e]), io_np),
                "Wd": _tile_w_down(prep(Wd[e]), io_np),
            }
        )

    res = run_bass_kernel_spmd(
        nc,
        in_maps,
        list(range(NCORES)),
        trace=TRACE,
        trace_cores=TRACE_CORES if TRACE else None,
    )
    LAST_RESULT = res

    out = np.zeros((T, H), dtype=np.float32)
    for e in range(E):
        idx = idx_list[e]
        dTe = np.asarray(res.results[e]["dT"], dtype=np.float32)  # [H, C]
        out[idx] += w_list[e][:, None] * dTe[:, : len(idx)].T
    return out
